# revision 7
# baseline (speedup 1.0000x reference)
"""Trainium2 Bass kernel for nn_DimensionShuffler.

Math: the reference collapses to  out = xg @ E  with
  xg = input[:, :, in_mapping]                      (host gather, B*S x IN)
  E  = in2( out2( mask * out1( in1(W2) ) ) )        (device, IN x OUT)
where W2 = (weight[out_mapping][:, in_mapping]).T and the four mixes are
block-diagonal 2x2 rotations built on-device from cos/sin of the permutation
scores.  All permutations are hoisted to host-side sharding (index-only);
all arithmetic (sin/cos, rotations, mask, matmuls) runs on device in fp32r
(TF32-like, full PE speed, ~1.5e-4 relative error).

Device pipeline per core, per OUT-chunk (v5):
  M1:  A = R_in1 @ W2        left-mult matmul per IN-128 chunk   (IN-p, OUT-f)
  Ta:  A2 = A^T              pure PE transpose                   (OUT-p, IN-f)
  M2:  B = R_out1 @ A2       left-mult per OUT-128 chunk
       evict fused with natural-layout mask multiply (DVE)
  M3:  C = R_out2 @ B        left-mult per OUT-128 chunk
  Tb:  D = C^T               pure PE transpose                   (IN-p, OUT-f)
  M4:  E = R_in2 @ D         left-mult per IN-128 chunk
  MAIN: out[m,n] = sum_p xgT[p,m]^T @ E[p,n]                     (rows-p, OUT-f)

SBUF layout: a logical (C*128, F) matrix is stored partition-folded as one
[128, C*F] tile; 128-row chunk c lives at columns [c*F:(c+1)*F].

Sharding: data-parallel over B*S rows (8 cores x 1024 rows); weight-side
tensors replicated.  Output written in natural (rows, OUT) layout; host
concatenates shards and applies the final out_mapping_reverse gather.
"""
import numpy as np

B, S, IN, OUT = 4, 2048, 1024, 4096
N_CORES = 8
ROWS = B * S // N_CORES          # 1024 rows per core

_COMPILED = {}


def build(mode="f32r", dims=(IN, OUT, ROWS), chunk=256, n_cores=N_CORES,
          debug_e=False):
    """Build + compile the Bass module. mode: 'f32r' | 'bf16' | 'f32'."""
    import concourse.bacc as bacc
    import concourse.mybir as mybir
    import concourse.tile as tile

    F32 = mybir.dt.float32
    F32R = mybir.dt.float32r
    BF16 = mybir.dt.bfloat16
    DT = {"f32r": F32R, "bf16": BF16, "f32": F32}[mode]
    AF = mybir.ActivationFunctionType
    ALU = mybir.AluOpType

    d_in, d_out, d_rows = dims
    P = 128
    NCH = d_out // chunk          # out-chunks
    NJ = chunk // P               # 128-subchunks per out-chunk
    NP = d_in // P                # in-chunks
    NOC = d_out // P              # out-128 chunks total
    NSI = max(1, d_in // 512)     # <=512-wide spans across d_in
    WI = min(d_in, 512)
    NSC = max(1, chunk // 512)    # <=512-wide spans across chunk
    WC = min(chunk, 512)
    NM = d_rows // P              # row tiles

    def fold(ap, p=P):
        # DRAM view (C*128, F) -> [128, C, F] partition-folded
        return ap.rearrange("(c p) f -> p c f", p=p)

    def cast_in(ap):
        return ap.bitcast(F32R) if DT == F32R else ap

    nc = bacc.Bacc("TRN2", target_bir_lowering=False, debug=False,
                   num_devices=n_cores)

    d_xgT = nc.dram_tensor("xgT", [d_in, d_rows], F32, kind="ExternalInput")
    d_w2 = nc.dram_tensor("w2", [d_in, d_out], F32, kind="ExternalInput")
    d_mask = nc.dram_tensor("mask", [d_out, d_in], F32, kind="ExternalInput")
    d_iang = nc.dram_tensor("in_ang", [P, NP], F32, kind="ExternalInput")
    d_oang = nc.dram_tensor("out_ang", [P, NOC], F32, kind="ExternalInput")
    d_eye = nc.dram_tensor("eye", [P, P], F32, kind="ExternalInput")
    d_sup = nc.dram_tensor("sup", [P, P], F32, kind="ExternalInput")
    d_sub = nc.dram_tensor("sub", [P, P], F32, kind="ExternalInput")
    d_pme = nc.dram_tensor("pm_even", [P, 1], F32, kind="ExternalInput")
    d_pmo = nc.dram_tensor("pm_odd", [P, 1], F32, kind="ExternalInput")
    d_o = nc.dram_tensor("out", [d_rows, d_out], F32, kind="ExternalOutput")
    d_e = None
    if debug_e:
        d_e = nc.dram_tensor("dbgE", [d_in, d_out], F32, kind="ExternalOutput")

    with tile.TileContext(nc) as tc:
        with (
            tc.tile_pool(name="const", bufs=1) as const,
            tc.tile_pool(name="rmat", bufs=1) as rmat,
            tc.tile_pool(name="xg", bufs=1) as xgp,
            tc.tile_pool(name="wio", bufs=2) as wio,
            tc.tile_pool(name="stg1", bufs=1) as stg1,
            tc.tile_pool(name="stg", bufs=2) as stg,
            tc.tile_pool(name="outp", bufs=3) as outp,
            tc.tile_pool(name="ps_w", bufs=2, space="PSUM") as ps_w,
            tc.tile_pool(name="ps_o", bufs=3, space="PSUM") as ps_o,
        ):
            # ---- constants ----
            t_eye = const.tile([P, P], F32)
            t_eyer = const.tile([P, P], DT)      # identity for transposes
            t_sup = const.tile([P, P], F32)
            t_sub = const.tile([P, P], F32)
            t_pme = const.tile([P, 1], F32)
            t_pmo = const.tile([P, 1], F32)
            nc.sync.dma_start(out=t_eye[:], in_=d_eye.ap())
            nc.sync.dma_start(out=t_sup[:], in_=d_sup.ap())
            nc.sync.dma_start(out=t_sub[:], in_=d_sub.ap())
            nc.sync.dma_start(out=t_pme[:], in_=d_pme.ap())
            nc.sync.dma_start(out=t_pmo[:], in_=d_pmo.ap())
            nc.vector.tensor_copy(t_eyer[:], t_eye[:])
            t_hpi = const.tile([P, 1], F32)
            nc.vector.memset(t_hpi[:], float(np.pi / 2))

            # ---- angles -> cos/sin ----
            t_iang = const.tile([P, NP], F32)
            t_oang = const.tile([P, NOC], F32)
            nc.sync.dma_start(out=t_iang[:], in_=d_iang.ap())
            nc.sync.dma_start(out=t_oang[:], in_=d_oang.ap())
            t_ic = const.tile([P, NP], F32)
            t_is = const.tile([P, NP], F32)
            t_oc = const.tile([P, NOC], F32)
            t_os = const.tile([P, NOC], F32)
            nc.scalar.activation(t_is[:], t_iang[:], AF.Sin)
            nc.scalar.activation(t_ic[:], t_iang[:], AF.Sin, bias=t_hpi[:])
            nc.scalar.activation(t_os[:], t_oang[:], AF.Sin)
            nc.scalar.activation(t_oc[:], t_oang[:], AF.Sin, bias=t_hpi[:])

            # ---- rotation matrices (lhsT form), kept resident ----
            # One [128, NMAT*128] tile per family.
            t_rm1 = rmat.tile([P, NP * P], DT)
            t_rm4 = rmat.tile([P, NP * P], DT)
            t_rm2 = rmat.tile([P, NOC * P], DT)
            t_rm3 = rmat.tile([P, NOC * P], DT)

            def build_R(dst, c_col, s_col, pattern):
                # pattern 'A': lhsT = [[c,-s],[s,c]]  (sup=-s@even, sub=+s@odd)
                # pattern 'B': lhsT = [[c,+s],[-s,c]] (sup=+s@even, sub=-s@odd)
                sgn = 1.0 if pattern == "B" else -1.0
                sv_sup = rmat.tile([P, 1], F32, tag="sv_sup")
                sv_sub = rmat.tile([P, 1], F32, tag="sv_sub")
                nc.vector.tensor_tensor(sv_sup[:], s_col, t_pme[:], ALU.mult)
                nc.vector.tensor_scalar_mul(sv_sup[:], sv_sup[:], sgn)
                nc.vector.tensor_tensor(sv_sub[:], s_col, t_pmo[:], ALU.mult)
                nc.vector.tensor_scalar_mul(sv_sub[:], sv_sub[:], -sgn)
                acc = rmat.tile([P, P], F32, tag="racc")
                nc.vector.tensor_scalar_mul(acc[:], t_eye[:], c_col)
                nc.vector.scalar_tensor_tensor(
                    acc[:], t_sup[:], sv_sup[:], acc[:], ALU.mult, ALU.add)
                nc.vector.scalar_tensor_tensor(
                    acc[:], t_sub[:], sv_sub[:], acc[:], ALU.mult, ALU.add)
                nc.vector.tensor_copy(dst, acc[:])

            for p in range(NP):
                build_R(t_rm1[:, p*P:(p+1)*P], t_ic[:, p:p+1], t_is[:, p:p+1], "A")
                build_R(t_rm4[:, p*P:(p+1)*P], t_ic[:, p:p+1], t_is[:, p:p+1], "B")
            for o in range(NOC):
                build_R(t_rm2[:, o*P:(o+1)*P], t_oc[:, o:o+1], t_os[:, o:o+1], "B")
                build_R(t_rm3[:, o*P:(o+1)*P], t_oc[:, o:o+1], t_os[:, o:o+1], "A")

            def RM1(p): return t_rm1[:, p*P:(p+1)*P]
            def RM4(p): return t_rm4[:, p*P:(p+1)*P]
            def RM2(o): return t_rm2[:, o*P:(o+1)*P]
            def RM3(o): return t_rm3[:, o*P:(o+1)*P]

            # ---- activations resident: [128, NP*d_rows] ----
            t_xgT = xgp.tile([P, NP * d_rows], DT)
            if DT == BF16:
                t_xf = xgp.tile([P, NP * d_rows], F32, tag="xf32")
                nc.sync.dma_start(out=t_xf[:], in_=fold(d_xgT.ap()))
                nc.vector.tensor_copy(t_xgT[:], t_xf[:])
            else:
                nc.sync.dma_start(
                    out=t_xgT[:].rearrange("p (c f) -> p c f", c=NP),
                    in_=cast_in(fold(d_xgT.ap())))

            def XG(p, m):
                return t_xgT[:, p*d_rows + m*P: p*d_rows + (m+1)*P]

            # ---- per out-chunk pipeline ----
            for n in range(NCH):
                oc = n * chunk
                # W2 chunk: logical (d_in, chunk) -> [128, NP*chunk]
                t_w2 = wio.tile([P, NP * chunk], DT, tag="w2c")
                nc.sync.dma_start(
                    out=t_w2[:].rearrange("p (c f) -> p c f", c=NP),
                    in_=cast_in(fold(d_w2.ap()[:, oc:oc+chunk])))
                # mask chunk: logical (chunk, d_in) -> [128, NJ*d_in]
                t_mk = wio.tile([P, NJ * d_in], F32, tag="mkc")
                nc.sync.dma_start(
                    out=t_mk[:].rearrange("p (c f) -> p c f", c=NJ),
                    in_=fold(d_mask.ap()[oc:oc+chunk, :]))

                # M1: A = R_in1 @ W2-chunk   (IN-p, chunk-f)
                a_sb = stg1.tile([P, NP * chunk], DT, tag="a_sb")
                for p in range(NP):
                    pa_t = ps_w.tile([P, max(chunk, d_in)], F32, tag="wp")
                    pa = pa_t[:, :chunk]
                    for h in range(NSC):
                        nc.tensor.matmul(pa[:, h*WC:(h+1)*WC], RM1(p),
                                         t_w2[:, p*chunk+h*WC:p*chunk+(h+1)*WC],
                                         start=True, stop=True)
                    nc.scalar.activation(a_sb[:, p*chunk:(p+1)*chunk], pa[:],
                                         AF.Copy)

                # Ta: A2 = A^T  (OUT-sub-p, IN-f) as [128, NJ*d_in]
                a2_sb = stg1.tile([P, NJ * d_in], DT, tag="a2_sb")
                for j in range(NJ):
                    pt_t = ps_w.tile([P, max(chunk, d_in)], DT, tag="wp")
                    pt = pt_t[:, :d_in]
                    for p in range(NP):
                        nc.tensor.transpose(
                            pt[:, p*P:(p+1)*P],
                            a_sb[:, p*chunk+j*P: p*chunk+(j+1)*P],
                            t_eyer[:])
                    nc.scalar.activation(a2_sb[:, j*d_in:(j+1)*d_in], pt[:],
                                         AF.Copy)

                # M2 + mask: B = mask * (R_out1 @ A2)
                b_sb = stg1.tile([P, NJ * d_in], DT, tag="b_sb")
                for j in range(NJ):
                    pb_t = ps_w.tile([P, max(chunk, d_in)], F32, tag="wp")
                    pb = pb_t[:, :d_in]
                    for h in range(NSI):
                        nc.tensor.matmul(pb[:, h*WI:(h+1)*WI],
                                         RM2((oc//P)+j),
                                         a2_sb[:, j*d_in+h*WI: j*d_in+(h+1)*WI],
                                         start=True, stop=True)
                    nc.vector.tensor_tensor(b_sb[:, j*d_in:(j+1)*d_in], pb[:],
                                            t_mk[:, j*d_in:(j+1)*d_in],
                                            ALU.mult)

                # M3: C = R_out2 @ B
                c_sb = stg1.tile([P, NJ * d_in], DT, tag="c_sb")
                for j in range(NJ):
                    pc_t = ps_w.tile([P, max(chunk, d_in)], F32, tag="wp")
                    pc = pc_t[:, :d_in]
                    for h in range(NSI):
                        nc.tensor.matmul(pc[:, h*WI:(h+1)*WI],
                                         RM3((oc//P)+j),
                                         b_sb[:, j*d_in+h*WI: j*d_in+(h+1)*WI],
                                         start=True, stop=True)
                    nc.scalar.activation(c_sb[:, j*d_in:(j+1)*d_in], pc[:],
                                         AF.Copy)

                # Tb: D = C^T  (IN-p, chunk-f) as [128, NP*chunk]
                d_sb = stg1.tile([P, NP * chunk], DT, tag="d_sb")
                for p in range(NP):
                    pt_t = ps_w.tile([P, max(chunk, d_in)], DT, tag="wp")
                    pt = pt_t[:, :chunk]
                    for j in range(NJ):
                        nc.tensor.transpose(
                            pt[:, j*P:(j+1)*P],
                            c_sb[:, j*d_in+p*P: j*d_in+(p+1)*P],
                            t_eyer[:])
                    nc.scalar.activation(d_sb[:, p*chunk:(p+1)*chunk], pt[:],
                                         AF.Copy)

                # M4: E = R_in2 @ D
                e_sb = stg.tile([P, NP * chunk], DT, tag="e_sb")
                for p in range(NP):
                    pe_t = ps_w.tile([P, max(chunk, d_in)], F32, tag="wp")
                    pe = pe_t[:, :chunk]
                    for h in range(NSC):
                        nc.tensor.matmul(pe[:, h*WC:(h+1)*WC], RM4(p),
                                         d_sb[:, p*chunk+h*WC: p*chunk+(h+1)*WC],
                                         start=True, stop=True)
                    nc.scalar.activation(e_sb[:, p*chunk:(p+1)*chunk], pe[:],
                                         AF.Copy)
                if debug_e:
                    nc.sync.dma_start(
                        out=fold(d_e.ap()[:, oc:oc+chunk]),
                        in_=e_sb[:].bitcast(F32).rearrange("p (c f) -> p c f", c=NP))

                # MAIN: out[m, n-chunk] = sum_p xgT[p,m]^T @ E[p, n-chunk]
                for m in range(NM):
                    for nn in range(NSC):
                        po = ps_o.tile([P, WC], F32, tag="pmain")
                        for p in range(NP):
                            nc.tensor.matmul(
                                po[:], XG(p, m),
                                e_sb[:, p*chunk+nn*WC: p*chunk+(nn+1)*WC],
                                start=(p == 0), stop=(p == NP - 1))
                        o_sb = outp.tile([P, WC], F32, tag="o_sb")
                        nc.scalar.activation(o_sb[:], po[:], AF.Copy)
                        nc.sync.dma_start(
                            out=d_o.ap()[m*P:(m+1)*P, oc+nn*WC:oc+(nn+1)*WC],
                            in_=o_sb[:])

    nc.compile()
    return nc


def host_prep(inputs, dims=(IN, OUT, ROWS), n_cores=N_CORES):
    d_in, d_out, d_rows = dims
    x = np.ascontiguousarray(inputs["input"], dtype=np.float32)
    W = np.ascontiguousarray(inputs["weight"], dtype=np.float32)
    mask = np.ascontiguousarray(inputs["mask"], dtype=np.float32)
    isc = np.asarray(inputs["in_permutation_scores"], np.float32)[:, 0]
    osc = np.asarray(inputs["out_permutation_scores"], np.float32)[:, 0]
    im = np.asarray(inputs["in_mapping"])
    om = np.asarray(inputs["out_mapping"])

    xg = x.reshape(-1, d_in)[:, im]                     # (B*S, IN) gathered
    Wg = W[om][:, im]                                   # (OUT, IN)
    W2 = np.ascontiguousarray(Wg.T)                     # (IN, OUT)

    iang = np.repeat(isc, 2).reshape(d_in // 128, 128).T.copy()
    oang = np.repeat(osc, 2).reshape(d_out // 128, 128).T.copy()

    eye = np.eye(128, dtype=np.float32)
    sup = np.eye(128, k=1, dtype=np.float32)
    sub = np.eye(128, k=-1, dtype=np.float32)
    pme = np.zeros((128, 1), np.float32); pme[0::2] = 1.0
    pmo = np.zeros((128, 1), np.float32); pmo[1::2] = 1.0

    shared = {
        "w2": W2, "mask": mask, "in_ang": iang, "out_ang": oang,
        "eye": eye, "sup": sup, "sub": sub, "pm_even": pme, "pm_odd": pmo,
    }
    in_maps = []
    for c in range(n_cores):
        xgT = np.ascontiguousarray(xg[c*d_rows:(c+1)*d_rows, :].T)
        in_maps.append({**shared, "xgT": xgT})
    return in_maps


def kernel(**inputs) -> np.ndarray:
    import sys
    if "/opt/trn_rl_repo" not in sys.path:
        sys.path.insert(0, "/opt/trn_rl_repo")
    from concourse.bass_utils import run_bass_kernel_spmd

    key = "main"
    if key not in _COMPILED:
        _COMPILED[key] = build()
    nc = _COMPILED[key]

    in_maps = host_prep(inputs)
    res = run_bass_kernel_spmd(nc, in_maps, core_ids=list(range(N_CORES)))

    omr = np.asarray(inputs["out_mapping_reverse"])
    s_pre = np.concatenate([res.results[c]["out"] for c in range(N_CORES)],
                           axis=0)
    out = s_pre.reshape(B, S, OUT)[:, :, omr]
    return np.ascontiguousarray(out)
